# revision 19
# baseline (speedup 1.0000x reference)
"""Trainium2 Bass kernel for per-node masked MLP (gnn_message_passing).

Reference computation (B=8192 batch, T=128 nodes, H=64 hidden, C=2 out):
    h   = leaky_relu(einsum('tij,jt,bj->bti', w0, adj, x) + b0)   adj = 1-eye
    h   = leaky_relu(einsum('tij,btj->bti', w1, h) + b1)
    out = einsum('tij,btj->bti', w2, h) + b2

Strategy: data-parallel over batch across 8 NeuronCores (1024 rows each).
Per core, all three layers are TensorE matmuls with the (t,i) axes on PSUM
partitions and batch streaming on the moving free dim. All weights are
preloaded into SBUF once.
  L0: [j=128, ti-tile=128] stationary per 128-wide ti block (self-loop mask
      folded into the weights host-side), fp32r.
  L1: block-diagonal [W1[2m].T (+) W1[2m+1].T] stationary per node pair,
      bf16 (weights + h0 quantized; rel err ~1e-3, budget 2e-2).
  L2: 128-wide bf16 stationary accumulating 32 node pairs into one PSUM
      bank (each pair owns a distinct 4-column strip).
PSUM evacuation (bias + leaky-relu, 2 layers x 64 pairs x 2 halves = 256
tiles of [128,512] per iteration) is the bottleneck when it rides a single
engine (ScalarE-only baseline: ACT busy 159us vs PE 89us).  v1 spreads it
over THREE engines, weighted by per-tile cost from the TRN2 cost model:
  ACT : 1-op Lrelu-with-bias            ~612 ns/tile
  DVE : add-bias -> bf16, max(v,.01v)   ~1252 ns/tile
  Pool: same 2-op sequence at 0.6 eff   ~1612 ns/tile
Greedy least-loaded assignment in program order yields ~84us of balanced
evacuation, hiding under the ~89us PE roofline.
"""

import sys

if "/opt/trn_rl_repo" not in sys.path:
    sys.path.insert(0, "/opt/trn_rl_repo")

import numpy as np

B = 8192
T = 128
H = 64
C = 2
N_CORES = 8
BC = B // N_CORES  # 1024 batch rows per core
M_TILES = 64  # 128-wide (t,i) tiles for L0 == node pairs for L1/L2
NEG = 0.01  # leaky_relu negative slope


def _split_sync_waits(nc, cap=1):
    """This container's walrus build encodes at most ~1 sync wait per
    instruction (setupSyncWait: "Too many sync wait commands"), while Tile's
    sem assignment freely attaches several. Post-pass: leave `cap` waits on
    each instruction and hoist the extras onto single-wait NOPs inserted
    just before it on the same engine (same-engine FIFO preserves
    semantics)."""
    from concourse import mybir

    ctr = [0]
    for f in nc.m.functions:
        for blk in f.blocks:
            new_list = []
            for ins in blk.instructions:
                si = getattr(ins, "sync_info", None)
                waits = list(si.on_wait) if si is not None and si.on_wait else []
                if len(waits) > cap:
                    keep = waits[:cap]
                    extra = waits[cap:]
                    for w in extra:
                        ctr[0] += 1
                        nop = mybir.InstNoOp(
                            name=f"{ins.name}-ws{ctr[0]}",
                            engine=ins.engine,
                            ins=[],
                            outs=[],
                            sync_info=mybir.SyncInfo(on_wait=[w], on_update=[]),
                        )
                        new_list.append(nop)
                    ins.sync_info = mybir.SyncInfo(
                        on_wait=keep, on_update=list(si.on_update or [])
                    )
                new_list.append(ins)
            blk.instructions[:] = new_list


def _lane_schedule(n_tiles, lane_usage):
    """Greedy engine-load-balancing assignment of evac tiles (in program
    order) to lanes. lane_usage: lane -> {engine: ns}. A lane may consume
    time on several engines (pipelined lanes). Returns list of lane names."""
    engines = {e for u in lane_usage.values() for e in u}
    load = {e: 0.0 for e in engines}
    out = []
    for _ in range(n_tiles):
        best, best_t = None, None
        for lane, usage in lane_usage.items():
            t = max(load[e] + c for e, c in usage.items())
            if best_t is None or t < best_t:
                best, best_t = lane, t
        for e, c in lane_usage[best].items():
            load[e] += c
        out.append(best)
    return out


def build_program(
    loop_R=None,
    repeat=1,
    lane_costs=(612.0, 658.0, 594.0, 806.0),  # act, dve_ts, dve_stt, pool_stt ns
    use_dve=True,
    use_pool=False,
    skew=2,
    ps_bufs=(3, 3),
    h_bufs=3,
    wait_cap=1,
    evac_width=1024,
):
    """Build the per-core Bass program.

    loop_R: wrap the body in a hardware For_i loop (wall-clock slope timing).
    """
    import concourse.bass as bass
    import concourse.tile as tile
    from concourse import mybir

    f32 = mybir.dt.float32
    f32r = mybir.dt.float32r
    bf16 = mybir.dt.bfloat16
    Alu = mybir.AluOpType
    Act = mybir.ActivationFunctionType

    # Per-lane engine usage (ns/tile).  GPSIMD cannot access PSUM on this
    # walrus build, so the "pool" lane is a DVE->Pool pipe: DVE reads the
    # PSUM tile (+bias -> bf16 tmp), Pool does the SBUF-only leaky max.
    act_c, dve_ts, dve_stt, pool_stt = lane_costs
    if evac_width == 1024:
        # one evac op per [128,1024] psum pair (2-bank ACT/DVE reads verified
        # on this runtime); per-op costs scale ~2x minus fixed overhead
        lane_usage = {"act": {"act": 996.0}}
        if use_dve:
            lane_usage["dve"] = {"dve": 1192.0 + 1127.0}
        n_units = 2 * M_TILES
    else:
        lane_usage = {"act": {"act": act_c}}
        if use_dve:
            lane_usage["dve"] = {"dve": dve_ts + dve_stt}
        if use_pool:
            lane_usage["pool"] = {"dve": dve_ts, "pool": pool_stt}
        n_units = 2 * M_TILES * 2
    schedule = _lane_schedule(n_units, lane_usage)

    nc = bass.Bass()
    xt_d = nc.dram_tensor("xt", [T, BC], f32r, kind="ExternalInput")
    w0_d = nc.dram_tensor("w0w", [T, M_TILES * T], f32r, kind="ExternalInput")
    w1_d = nc.dram_tensor("w1w", [T, M_TILES * T], bf16, kind="ExternalInput")
    w2_d = nc.dram_tensor("w2w", [T, M_TILES * T], bf16, kind="ExternalInput")
    b0_d = nc.dram_tensor("b0s", [T, M_TILES], f32, kind="ExternalInput")
    b1_d = nc.dram_tensor("b1s", [T, M_TILES], f32, kind="ExternalInput")
    b2_d = nc.dram_tensor("b2s", [T, 2], f32, kind="ExternalInput")
    out_d = nc.dram_tensor("out", [2 * T, BC], f32, kind="ExternalOutput")

    wide = evac_width == 1024
    with tile.TileContext(nc) as tc:
        with (
            tc.tile_pool(name="const", bufs=1) as cp,
            tc.tile_pool(name="h0p", bufs=h_bufs) as h0p,
            tc.tile_pool(name="h1p", bufs=h_bufs) as h1p,
            tc.tile_pool(name="tmpd", bufs=3) as tmpd,
            tc.tile_pool(name="tmpg", bufs=3) as tmpg,
            tc.tile_pool(name="outp", bufs=2) as outp,
            # wide mode: psA [T,1024] tiles (2 banks) shared by L0+L1 (6
            # banks) + psCa [T,1024] L2 accumulator (2 banks) = 8 banks.
            # narrow mode: psA/psB [T,512] + psCa/psCb as in the baseline.
            tc.tile_pool(
                name="psA",
                bufs=3 if wide else ps_bufs[0],
                space=bass.MemorySpace.PSUM,
            ) as psA,
            tc.tile_pool(
                name="psB", bufs=ps_bufs[1], space=bass.MemorySpace.PSUM
            ) as psB,
            tc.tile_pool(name="psCa", bufs=1, space=bass.MemorySpace.PSUM) as psCa,
            tc.tile_pool(name="psCb", bufs=1, space=bass.MemorySpace.PSUM) as psCb,
        ):
            # ---- resident tensors (loaded once) ----
            xtt = cp.tile([T, BC], f32r, tag="xt")
            nc.sync.dma_start(xtt[:], xt_d[:])
            w0sb = cp.tile([T, M_TILES * T], f32r, tag="w0w")
            nc.sync.dma_start(w0sb[:], w0_d[:])
            w1sb = cp.tile([T, M_TILES * T], bf16, tag="w1w")
            nc.sync.dma_start(w1sb[:], w1_d[:])
            w2sb = cp.tile([T, M_TILES * T], bf16, tag="w2w")
            nc.sync.dma_start(w2sb[:], w2_d[:])
            b0t = cp.tile([T, M_TILES], f32, tag="b0")
            nc.sync.dma_start(b0t[:], b0_d[:])
            b1t = cp.tile([T, M_TILES], f32, tag="b1")
            nc.sync.dma_start(b1t[:], b1_d[:])
            b2t = cp.tile([T, 2], f32, tag="b2")
            nc.sync.dma_start(b2t[:], b2_d[:])

            ctr = [0]

            def evac_leaky(dst, ps, bias_col):
                w = dst.shape[-1]
                lane = schedule[ctr[0] % len(schedule)]
                ctr[0] += 1
                if lane == "act":
                    nc.scalar.activation(
                        dst, ps, Act.Lrelu, bias=bias_col, scale=1.0, alpha=NEG
                    )
                else:
                    v = (tmpd if lane == "dve" else tmpg).tile(
                        [T, evac_width], bf16, tag="t1"
                    )
                    # v = ps + bias  (scalar2 slot carries the per-partition
                    # col; DVE is the only non-ACT engine that may read PSUM)
                    nc.vector.tensor_scalar(
                        v[:, 0:w], ps, 0.0, bias_col, op0=Alu.add, op1=Alu.add
                    )
                    # dst = max(v, 0.01*v) == leaky(v), SBUF-only
                    eng = nc.vector if lane == "dve" else nc.gpsimd
                    eng.scalar_tensor_tensor(
                        dst, v[:, 0:w], NEG, v[:, 0:w], op0=Alu.mult, op1=Alu.max
                    )

            state = {}

            def stage_l0(m):
                w0t = w0sb[:, T * m : T * (m + 1)]
                h0 = h0p.tile([T, BC], bf16, tag="h0")
                if wide:
                    ps0 = psA.tile([T, 1024], f32, tag="ps")
                    nc.tensor.matmul(
                        ps0[:, 0:512], w0t, xtt[:, 0:512], start=True, stop=True
                    )
                    nc.tensor.matmul(
                        ps0[:, 512:1024], w0t, xtt[:, 512:1024], start=True, stop=True
                    )
                    evac_leaky(h0[:, 0:1024], ps0[:, 0:1024], b0t[:, m : m + 1])
                else:
                    ps0a = psA.tile([T, 512], f32, tag="ps0")
                    nc.tensor.matmul(ps0a[:], w0t, xtt[:, 0:512], start=True, stop=True)
                    ps0b = psA.tile([T, 512], f32, tag="ps0")
                    nc.tensor.matmul(
                        ps0b[:], w0t, xtt[:, 512:1024], start=True, stop=True
                    )
                    evac_leaky(h0[:, 0:512], ps0a[:], b0t[:, m : m + 1])
                    evac_leaky(h0[:, 512:1024], ps0b[:], b0t[:, m : m + 1])
                state[("h0", m)] = h0

            def stage_l12(m):
                g, mq = divmod(m, 32)
                w1t = w1sb[:, T * m : T * (m + 1)]
                w2t = w2sb[:, T * m : T * (m + 1)]
                h0 = state.pop(("h0", m))
                h1 = h1p.tile([T, BC], bf16, tag="h1")
                if mq == 0:
                    if wide:
                        ps2 = psCa.tile([T, 1024], f32, tag="ps2")
                        state["ps2"] = (ps2[:, 0:512], ps2[:, 512:1024], ps2)
                    else:
                        ps2a = psCa.tile([T, 512], f32, tag="ps2a")
                        ps2b = psCb.tile([T, 512], f32, tag="ps2b")
                        state["ps2"] = (ps2a[:], ps2b[:], None)
                if wide:
                    ps1 = psA.tile([T, 1024], f32, tag="ps")
                    nc.tensor.matmul(
                        ps1[:, 0:512], w1t, h0[:, 0:512], start=True, stop=True
                    )
                    nc.tensor.matmul(
                        ps1[:, 512:1024], w1t, h0[:, 512:1024], start=True, stop=True
                    )
                    evac_leaky(h1[:, 0:1024], ps1[:, 0:1024], b1t[:, m : m + 1])
                else:
                    ps1a = psB.tile([T, 512], f32, tag="ps1")
                    nc.tensor.matmul(ps1a[:], w1t, h0[:, 0:512], start=True, stop=True)
                    ps1b = psB.tile([T, 512], f32, tag="ps1")
                    nc.tensor.matmul(
                        ps1b[:], w1t, h0[:, 512:1024], start=True, stop=True
                    )
                    evac_leaky(h1[:, 0:512], ps1a[:], b1t[:, m : m + 1])
                    evac_leaky(h1[:, 512:1024], ps1b[:], b1t[:, m : m + 1])
                ps2a, ps2b, ps2w = state["ps2"]
                nc.tensor.matmul(
                    ps2a, w2t, h1[:, 0:512], start=(mq == 0), stop=(mq == 31)
                )
                nc.tensor.matmul(
                    ps2b, w2t, h1[:, 512:1024], start=(mq == 0), stop=(mq == 31)
                )
                if mq == 31:
                    if wide:
                        oa = outp.tile([T, 1024], f32, tag="oa")
                        nc.scalar.activation(
                            oa[:], ps2w[:, 0:1024], Act.Identity,
                            bias=b2t[:, g : g + 1],
                        )
                        nc.sync.dma_start(
                            out_d[128 * g : 128 * (g + 1), 0:1024], oa[:]
                        )
                    else:
                        oa = outp.tile([T, 512], f32, tag="oa")
                        nc.scalar.activation(
                            oa[:], ps2a, Act.Identity, bias=b2t[:, g : g + 1]
                        )
                        nc.sync.dma_start(out_d[128 * g : 128 * (g + 1), 0:512], oa[:])
                        ob = outp.tile([T, 512], f32, tag="ob")
                        nc.scalar.activation(
                            ob[:], ps2b, Act.Identity, bias=b2t[:, g : g + 1]
                        )
                        nc.sync.dma_start(
                            out_d[128 * g : 128 * (g + 1), 512:1024], ob[:]
                        )

            def body(_iv=None):
                S = int(skew)
                for m in range(M_TILES + S):
                    if m < M_TILES:
                        stage_l0(m)
                    if m >= S:
                        stage_l12(m - S)

            if loop_R is None:
                for _ in range(repeat):
                    body()
            else:
                with tc.For_i(0, loop_R, 1) as iv:
                    body(iv)

    _split_sync_waits(nc, cap=wait_cap)
    return nc


def prep_inputs(x, w0, b0, w1, b1, w2, b2):
    """Host-side reshuffle of the full inputs into the per-core tensors."""
    import ml_dtypes

    bf = ml_dtypes.bfloat16
    x = np.ascontiguousarray(np.asarray(x, dtype=np.float32))
    w0 = np.asarray(w0, dtype=np.float32)
    b0 = np.asarray(b0, dtype=np.float32)
    w1 = np.asarray(w1, dtype=np.float32)
    b1 = np.asarray(b1, dtype=np.float32)
    w2 = np.asarray(w2, dtype=np.float32)
    b2 = np.asarray(b2, dtype=np.float32)

    # L0 stationaries: mask self-loop; [j, (m p)] with column 128m+p -> ti
    w0m = w0.copy()
    w0m[np.arange(T), :, np.arange(T)] = 0.0
    w0w = np.ascontiguousarray(w0m.transpose(2, 0, 1).reshape(T, T * H))

    # L1 stationaries: block-diag of the pair's transposed weights (bf16)
    w1T = w1.transpose(0, 2, 1)  # [t, i_in, i_out]
    w1s = np.zeros((M_TILES, T, T), np.float32)
    w1s[:, :H, :H] = w1T[0::2]
    w1s[:, H:, H:] = w1T[1::2]
    w1w = np.ascontiguousarray(
        w1s.transpose(1, 0, 2).reshape(T, M_TILES * T).astype(bf)
    )

    # L2 stationaries: pair m owns columns 4*(m%32) .. +4 (bf16)
    w2T = w2.transpose(0, 2, 1)  # [t, i, c]
    w2s = np.zeros((M_TILES, T, T), np.float32)
    for m in range(M_TILES):
        col = 4 * (m % 32)
        w2s[m, :H, col : col + C] = w2T[2 * m]
        w2s[m, H:, col + C : col + 2 * C] = w2T[2 * m + 1]
    w2w = np.ascontiguousarray(
        w2s.transpose(1, 0, 2).reshape(T, M_TILES * T).astype(bf)
    )

    b0s = np.ascontiguousarray(b0.reshape(-1).reshape(M_TILES, T).T)
    b1s = np.ascontiguousarray(b1.reshape(-1).reshape(M_TILES, T).T)
    b2s = np.ascontiguousarray(b2.reshape(-1).reshape(2, T).T)

    shared = {
        "w0w": w0w, "w1w": w1w, "w2w": w2w,
        "b0s": b0s, "b1s": b1s, "b2s": b2s,
    }
    in_maps = []
    for c in range(N_CORES):
        xt_c = np.ascontiguousarray(x[c * BC : (c + 1) * BC].T)  # [128, BC]
        in_maps.append({"xt": xt_c, **shared})
    return in_maps


def gather_output(results):
    """results: list of per-core {"out": [256, BC]} -> full [B, T, C]."""
    parts = []
    for c in range(N_CORES):
        o = np.asarray(results[c]["out"])  # [2T, BC], row r = t*2+c
        parts.append(o.reshape(T, C, BC).transpose(2, 0, 1))
    return np.ascontiguousarray(np.concatenate(parts, axis=0))


_NC_CACHE = {}

BEST_CONFIG = dict()


def kernel(x, w0, b0, w1, b1, w2, b2):
    from concourse.bass_utils import run_bass_kernel_spmd

    if "nc" not in _NC_CACHE:
        _NC_CACHE["nc"] = build_program(**BEST_CONFIG)
    nc = _NC_CACHE["nc"]
    in_maps = prep_inputs(x, w0, b0, w1, b1, w2, b2)
    res = run_bass_kernel_spmd(nc, in_maps, core_ids=list(range(N_CORES)))
    return gather_output(res.results)


# revision 52
# speedup vs baseline: 1.5449x; 1.5449x over previous
"""Trainium2 Bass kernel for per-node masked MLP (gnn_message_passing).

Reference computation (B=8192 batch, T=128 nodes, H=64 hidden, C=2 out):
    h   = leaky_relu(einsum('tij,jt,bj->bti', w0, adj, x) + b0)   adj = 1-eye
    h   = leaky_relu(einsum('tij,btj->bti', w1, h) + b1)
    out = einsum('tij,btj->bti', w2, h) + b2

Strategy: data-parallel over batch across 8 NeuronCores (1024 rows each).
Per core, all three layers are TensorE matmuls with the (t,i) axes on PSUM
partitions and batch streaming on the moving free dim (PE floor measured at
81.9us/iter = the 2.4GHz roofline). All weights preloaded into SBUF once,
chunked+spread across DGE queues so first compute starts ~6us in.
  L0: [j=128, ti-tile=128] stationary per 128-wide ti block (self-loop mask
      folded into the weights host-side), fp32r.
  L1: block-diagonal [W1[2m].T (+) W1[2m+1].T] stationary per node pair,
      bf16 (weights + h0/h1 quantized; rel err ~3.6e-3, budget 2e-2).
  L2: 128-wide bf16 stationary accumulating 32 node pairs into one [T,1024]
      PSUM accumulator (each pair owns a distinct 4-column strip); its
      matmuls are issued one macro-step late (l2_delay) at the head of each
      step so h1 evacuation latency stays off the in-order PE queue.
PSUM evacuation (bias + leaky-relu, 16.8M elem/core/iter) is the bottleneck:
it is split over the only two engines this walrus build allows to read PSUM
(GPSIMD cannot; custom fused DVE ops are rejected; DVE ops may read PSUM
only once, killing 1-op leaky variants):
  ACT : 1-op Lrelu-with-bias on [128,1024] 2-bank reads   (~1.1us, ~70%)
  DVE : add-bias -> bf16 tmp, then max(v,.01v)            (~2.7us, ~30%)
Units are greedily load-balanced in program order.  Measured ~111us/iter
(baseline 167us), between the 103us evac roofline and the 82us PE roofline.
"""

import sys

if "/opt/trn_rl_repo" not in sys.path:
    sys.path.insert(0, "/opt/trn_rl_repo")

import numpy as np

B = 8192
T = 128
H = 64
C = 2
N_CORES = 8
BC = B // N_CORES  # 1024 batch rows per core
M_TILES = 64  # 128-wide (t,i) tiles for L0 == node pairs for L1/L2
NEG = 0.01  # leaky_relu negative slope


def _split_sync_waits(nc, cap=1):
    """This container's walrus build encodes at most ~1 sync wait per
    instruction (setupSyncWait: "Too many sync wait commands"), while Tile's
    sem assignment freely attaches several. Post-pass: leave `cap` waits on
    each instruction and hoist the extras onto single-wait NOPs inserted
    just before it on the same engine (same-engine FIFO preserves
    semantics)."""
    from concourse import mybir

    ctr = [0]
    for f in nc.m.functions:
        for blk in f.blocks:
            new_list = []
            for ins in blk.instructions:
                si = getattr(ins, "sync_info", None)
                waits = list(si.on_wait) if si is not None and si.on_wait else []
                if len(waits) > cap:
                    keep = waits[:cap]
                    extra = waits[cap:]
                    for w in extra:
                        ctr[0] += 1
                        nop = mybir.InstNoOp(
                            name=f"{ins.name}-ws{ctr[0]}",
                            engine=ins.engine,
                            ins=[],
                            outs=[],
                            sync_info=mybir.SyncInfo(on_wait=[w], on_update=[]),
                        )
                        new_list.append(nop)
                    ins.sync_info = mybir.SyncInfo(
                        on_wait=keep, on_update=list(si.on_update or [])
                    )
                new_list.append(ins)
            blk.instructions[:] = new_list


def _lane_schedule(n_tiles, lane_usage):
    """Greedy engine-load-balancing assignment of evac tiles (in program
    order) to lanes. lane_usage: lane -> {engine: ns}. A lane may consume
    time on several engines (pipelined lanes). Returns list of lane names."""
    engines = {e for u in lane_usage.values() for e in u}
    load = {e: 0.0 for e in engines}
    out = []
    for _ in range(n_tiles):
        best, best_t = None, None
        for lane, usage in lane_usage.items():
            t = max(load[e] + c for e, c in usage.items())
            if best_t is None or t < best_t:
                best, best_t = lane, t
        for e, c in lane_usage[best].items():
            load[e] += c
        out.append(best)
    return out


def build_program(
    loop_R=None,
    repeat=1,
    lane_costs=(1100.0, 1450.0, 1250.0, 806.0),  # act, dve_ts, dve_stt, pool_stt ns
    use_dve=True,
    use_pool=False,
    skew=2,
    ps_bufs=(3, 3),
    h_bufs=3,
    wait_cap=1,
    evac_width=1024,
    h_dtype="bf16",
    wide_bufs=(3, 1),
    l2_delay=0,
    lane_policy="greedy",
    flush_first=False,
    probe=None,
    pe_bias=None,  # (n_act, n_dve1, n_dve2) unit counts for PE-biased mode
):
    """Build the per-core Bass program.

    loop_R: wrap the body in a hardware For_i loop (wall-clock slope timing).
    """
    import concourse.bass as bass
    import concourse.tile as tile
    from concourse import mybir

    f32 = mybir.dt.float32
    f32r = mybir.dt.float32r
    bf16 = mybir.dt.bfloat16
    Alu = mybir.AluOpType
    Act = mybir.ActivationFunctionType
    mixed = h_dtype == "mixed"
    hdt = bf16 if h_dtype == "bf16" else f32r
    if mixed:
        assert evac_width == 1024, "mixed h_dtype requires wide evac"

    # Per-lane engine usage (ns/tile).  GPSIMD cannot access PSUM on this
    # walrus build, so the "pool" lane is a DVE->Pool pipe: DVE reads the
    # PSUM tile (+bias -> bf16 tmp), Pool does the SBUF-only leaky max.
    act_c, dve_ts, dve_stt, pool_stt = lane_costs
    if evac_width == 1024:
        # one evac op per [128,1024] psum pair (2-bank ACT/DVE reads verified
        # on this runtime); HW microbench: ACT-bf16 ~2050, DVE 2-op ~2712
        lane_usage = {"act": {"act": act_c}}
        if use_dve:
            lane_usage["dve"] = {"dve": dve_ts + dve_stt}
        n_units = 2 * M_TILES
        if pe_bias is not None:
            n_act, n_dve1, n_dve2 = pe_bias
            assert n_act + n_dve1 + n_dve2 == n_units
            quota = {"act": n_act, "dve1": n_dve1, "dve": n_dve2}
            used = {k: 0 for k in quota}
            sched = []
            for i in range(n_units):
                best, best_err = None, None
                for k, q in quota.items():
                    if q == 0:
                        continue
                    err = (used[k] + 1) / q
                    if best_err is None or err < best_err:
                        best, best_err = k, err
                used[best] += 1
                sched.append(best)
            schedule = sched
        elif lane_policy == "l1dve":
            # L0 units always ACT; L1 units greedily ACT/DVE by load
            S = int(skew)
            kinds = []
            for m in range(M_TILES + S):
                if m < M_TILES:
                    kinds.append("l0")
                if m >= S:
                    kinds.append("l1")
            load = {"act": 0.0, "dve": 0.0}
            sched = []
            for k in kinds:
                if k == "l0" or not use_dve:
                    lane = "act"
                else:
                    ta = load["act"] + act_c
                    td = load["dve"] + dve_ts + dve_stt
                    lane = "act" if ta <= td else "dve"
                load[lane] += act_c if lane == "act" else dve_ts + dve_stt
                sched.append(lane)
            schedule = sched
        else:
            schedule = None
    else:
        lane_usage = {"act": {"act": act_c}}
        if use_dve:
            lane_usage["dve"] = {"dve": dve_ts + dve_stt}
        if use_pool:
            lane_usage["pool"] = {"dve": dve_ts, "pool": pool_stt}
        n_units = 2 * M_TILES * 2
        schedule = None
    if schedule is None:
        schedule = _lane_schedule(n_units, lane_usage)

    nc = bass.Bass()
    xt_d = nc.dram_tensor("xt", [T, BC], f32r, kind="ExternalInput")
    w0_d = nc.dram_tensor("w0w", [T, M_TILES * T], f32r, kind="ExternalInput")
    if mixed:
        w1_d = nc.dram_tensor("w1w", [T, M_TILES * T], f32r, kind="ExternalInput")
        w2_d = nc.dram_tensor("w2w", [T, M_TILES * T], f32r, kind="ExternalInput")
        w1b_d = nc.dram_tensor("w1wb", [T, M_TILES * T], bf16, kind="ExternalInput")
        w2b_d = nc.dram_tensor("w2wb", [T, M_TILES * T], bf16, kind="ExternalInput")
    else:
        w1_d = nc.dram_tensor("w1w", [T, M_TILES * T], hdt, kind="ExternalInput")
        w2_d = nc.dram_tensor("w2w", [T, M_TILES * T], hdt, kind="ExternalInput")
    b0_d = nc.dram_tensor("b0s", [T, M_TILES], f32, kind="ExternalInput")
    if pe_bias is not None:
        b0r_d = nc.dram_tensor("b0r", [1, M_TILES * T], f32r, kind="ExternalInput")
        b1r_d = nc.dram_tensor("b1r", [1, M_TILES * T], f32r, kind="ExternalInput")
    b1_d = nc.dram_tensor("b1s", [T, M_TILES], f32, kind="ExternalInput")
    b2_d = nc.dram_tensor("b2s", [T, 2], f32, kind="ExternalInput")
    out_d = nc.dram_tensor("out", [2 * T, BC], f32, kind="ExternalOutput")

    wide = evac_width == 1024
    with tile.TileContext(nc) as tc:
        with (
            tc.tile_pool(name="const", bufs=1) as cp,
            tc.tile_pool(name="h0p", bufs=h_bufs) as h0p,
            tc.tile_pool(name="h1p", bufs=h_bufs) as h1p,
            tc.tile_pool(name="tmpd", bufs=4) as tmpd,
            tc.tile_pool(name="tmpg", bufs=3) as tmpg,
            tc.tile_pool(name="outp", bufs=2) as outp,
            # wide mode: psA [T,1024] tiles (2 banks) shared by L0+L1 (6
            # banks) + psCa [T,1024] L2 accumulator (2 banks) = 8 banks.
            # narrow mode: psA/psB [T,512] + psCa/psCb as in the baseline.
            tc.tile_pool(
                name="psA",
                bufs=wide_bufs[0] if wide else ps_bufs[0],
                space=bass.MemorySpace.PSUM,
            ) as psA,
            tc.tile_pool(
                name="psB", bufs=ps_bufs[1], space=bass.MemorySpace.PSUM
            ) as psB,
            tc.tile_pool(
                name="psCa",
                bufs=wide_bufs[1] if wide else 1,
                space=bass.MemorySpace.PSUM,
            ) as psCa,
            tc.tile_pool(name="psCb", bufs=1, space=bass.MemorySpace.PSUM) as psCb,
        ):
            # ---- resident tensors (loaded once) ----
            # head-time choreography: tiny bias columns first (they gate the
            # first evacuation), x on the ACT DGE queue in parallel with w0
            # chunk 0 on the SP queue.
            b0t = cp.tile([T, M_TILES], f32, tag="b0")
            nc.sync.dma_start(b0t[:], b0_d[:])
            b1t = cp.tile([T, M_TILES], f32, tag="b1")
            nc.sync.dma_start(b1t[:], b1_d[:])
            b2t = cp.tile([T, 2], f32, tag="b2")
            nc.sync.dma_start(b2t[:], b2_d[:])
            xtt = cp.tile([T, BC], f32r, tag="xt")
            nc.scalar.dma_start(xtt[:], xt_d[:])
            # w0 in 8 chunks so the first L0 matmuls start before the
            # whole 4MB stationary set has landed (single-shot head time)
            W0CH = 8
            w0chunks = []
            for c in range(W0CH):
                w0c = cp.tile([T, (M_TILES // W0CH) * T], f32r, tag=f"w0w{c}",
                              name=f"w0c{c}")
                w0chunks.append(w0c)
            mpc0 = M_TILES // W0CH

            def load_w0(c):
                nc.sync.dma_start(
                    w0chunks[c][:],
                    w0_d[:, (M_TILES // W0CH) * T * c : (M_TILES // W0CH) * T * (c + 1)],
                )

            load_w0(0)
            WCH = 2
            w1chunks, w2chunks = [], []
            for c in range(WCH):
                sl = slice((M_TILES // WCH) * T * c, (M_TILES // WCH) * T * (c + 1))
                w1c = cp.tile([T, (M_TILES // WCH) * T], f32r if mixed else hdt,
                              tag=f"w1w{c}", name=f"w1c{c}")
                nc.scalar.dma_start(w1c[:], w1_d[:, sl])
                w1chunks.append(w1c)
                w2c = cp.tile([T, (M_TILES // WCH) * T], f32r if mixed else hdt,
                              tag=f"w2w{c}", name=f"w2c{c}")
                nc.scalar.dma_start(w2c[:], w2_d[:, sl])
                w2chunks.append(w2c)
            mpc = M_TILES // WCH
            if mixed:
                w1sb16 = cp.tile([T, M_TILES * T], bf16, tag="w1wb")
                nc.sync.dma_start(w1sb16[:], w1b_d[:])
                w2sb16 = cp.tile([T, M_TILES * T], bf16, tag="w2wb")
                nc.sync.dma_start(w2sb16[:], w2b_d[:])
            for c in range(1, W0CH):
                load_w0(c)
            if pe_bias is not None:
                b0rt = cp.tile([1, M_TILES * T], f32r, tag="b0r")
                nc.sync.dma_start(b0rt[:], b0r_d[:])
                b1rt = cp.tile([1, M_TILES * T], f32r, tag="b1r")
                nc.sync.dma_start(b1rt[:], b1r_d[:])
                ones = cp.tile([1, 512], f32r, tag="ones")
                nc.gpsimd.memset(ones[:].bitcast(f32), 1.0)

            if probe == "pe_only":
                h0fix = cp.tile([T, BC], hdt, tag="h0fix")
                nc.gpsimd.memset(h0fix[:].bitcast(mybir.dt.float32 if hdt is f32r else hdt), 0.125)
                h1fix = cp.tile([T, BC], hdt, tag="h1fix")
                nc.gpsimd.memset(h1fix[:].bitcast(mybir.dt.float32 if hdt is f32r else hdt), 0.125)

            ctr = [0]

            def evac_leaky(dst, ps, bias_col):
                if probe == "pe_only":
                    ctr[0] += 1
                    return
                w = dst.shape[-1]
                lane = schedule[ctr[0] % len(schedule)]
                ctr[0] += 1
                if lane == "act":
                    nc.scalar.activation(
                        dst, ps, Act.Lrelu, bias=bias_col, scale=1.0, alpha=NEG
                    )
                elif lane == "dve1":
                    # psum already contains z + b (PE bias matmul); one DVE op
                    nc.vector.scalar_tensor_tensor(
                        dst, ps, NEG, ps, op0=Alu.mult, op1=Alu.max
                    )
                else:
                    v = (tmpd if lane == "dve" else tmpg).tile(
                        [T, evac_width], hdt, tag="t1"
                    )
                    # v = ps + bias  (scalar2 slot carries the per-partition
                    # col; DVE is the only non-ACT engine that may read PSUM)
                    nc.vector.tensor_scalar(
                        v[:, 0:w], ps, 0.0, bias_col, op0=Alu.add, op1=Alu.add
                    )
                    # dst = max(v, 0.01*v) == leaky(v), SBUF-only
                    eng = nc.vector if lane == "dve" else nc.gpsimd
                    eng.scalar_tensor_tensor(
                        dst, v[:, 0:w], NEG, v[:, 0:w], op0=Alu.mult, op1=Alu.max
                    )

            state = {}

            def unit_dt():
                lane = schedule[ctr[0] % len(schedule)]
                return (f32r if lane == "act" else bf16) if mixed else hdt

            def peek_lane():
                return schedule[ctr[0] % len(schedule)]

            def inject_bias(ps, brow, m):
                nc.tensor.matmul(
                    ps[:, 0:512], brow[0:1, T * m : T * (m + 1)], ones[0:1, 0:512],
                    start=True, stop=False,
                )
                nc.tensor.matmul(
                    ps[:, 512:1024], brow[0:1, T * m : T * (m + 1)], ones[0:1, 0:512],
                    start=True, stop=False,
                )

            def stage_l0(m):
                w0t = w0chunks[m // mpc0][:, T * (m % mpc0) : T * (m % mpc0 + 1)]
                dt0 = unit_dt()
                h0 = h0fix if probe == "pe_only" else h0p.tile([T, BC], dt0, tag="h0")
                if wide:
                    ps0 = psA.tile([T, 1024], f32, tag="ps")
                    biased = pe_bias is not None and peek_lane() == "dve1"
                    if biased:
                        inject_bias(ps0, b0rt, m)
                    nc.tensor.matmul(
                        ps0[:, 0:512], w0t, xtt[:, 0:512],
                        start=not biased, stop=True,
                    )
                    nc.tensor.matmul(
                        ps0[:, 512:1024], w0t, xtt[:, 512:1024],
                        start=not biased, stop=True,
                    )
                    evac_leaky(h0[:, 0:1024], ps0[:, 0:1024], b0t[:, m : m + 1])
                else:
                    ps0a = psA.tile([T, 512], f32, tag="ps0")
                    nc.tensor.matmul(ps0a[:], w0t, xtt[:, 0:512], start=True, stop=True)
                    ps0b = psA.tile([T, 512], f32, tag="ps0")
                    nc.tensor.matmul(
                        ps0b[:], w0t, xtt[:, 512:1024], start=True, stop=True
                    )
                    evac_leaky(h0[:, 0:512], ps0a[:], b0t[:, m : m + 1])
                    evac_leaky(h0[:, 512:1024], ps0b[:], b0t[:, m : m + 1])
                state[("h0", m)] = (h0, dt0)

            def stage_l12(m):
                g, mq = divmod(m, 32)
                h0, dt0 = state.pop(("h0", m))
                dt1 = unit_dt()
                assert not mixed, "mixed mode incompatible with chunked weights"
                w1t = w1chunks[m // mpc][:, T * (m % mpc) : T * (m % mpc + 1)]
                w2t = w2chunks[m // mpc][:, T * (m % mpc) : T * (m % mpc + 1)]
                h1 = h1fix if probe == "pe_only" else h1p.tile([T, BC], dt1, tag="h1")
                if mq == 0:
                    if wide:
                        ps2 = psCa.tile([T, 1024], f32, tag="ps2")
                        state["ps2"] = (ps2[:, 0:512], ps2[:, 512:1024], ps2)
                    else:
                        ps2a = psCa.tile([T, 512], f32, tag="ps2a")
                        ps2b = psCb.tile([T, 512], f32, tag="ps2b")
                        state["ps2"] = (ps2a[:], ps2b[:], None)
                if wide:
                    ps1 = psA.tile([T, 1024], f32, tag="ps")
                    biased = pe_bias is not None and peek_lane() == "dve1"
                    if biased:
                        inject_bias(ps1, b1rt, m)
                    nc.tensor.matmul(
                        ps1[:, 0:512], w1t, h0[:, 0:512],
                        start=not biased, stop=True,
                    )
                    nc.tensor.matmul(
                        ps1[:, 512:1024], w1t, h0[:, 512:1024],
                        start=not biased, stop=True,
                    )
                    evac_leaky(h1[:, 0:1024], ps1[:, 0:1024], b1t[:, m : m + 1])
                else:
                    ps1a = psB.tile([T, 512], f32, tag="ps1")
                    nc.tensor.matmul(ps1a[:], w1t, h0[:, 0:512], start=True, stop=True)
                    ps1b = psB.tile([T, 512], f32, tag="ps1")
                    nc.tensor.matmul(
                        ps1b[:], w1t, h0[:, 512:1024], start=True, stop=True
                    )
                    evac_leaky(h1[:, 0:512], ps1a[:], b1t[:, m : m + 1])
                    evac_leaky(h1[:, 512:1024], ps1b[:], b1t[:, m : m + 1])
                def do_l2(h1=h1, w2t=w2t, g=g, mq=mq, trip=state["ps2"]):
                    ps2a, ps2b, ps2w = trip
                    nc.tensor.matmul(
                        ps2a, w2t, h1[:, 0:512], start=(mq == 0), stop=(mq == 31)
                    )
                    nc.tensor.matmul(
                        ps2b, w2t, h1[:, 512:1024], start=(mq == 0), stop=(mq == 31)
                    )
                    finish_l2(ps2a, ps2b, ps2w, g, mq)

                if l2_delay == 0:
                    do_l2()
                else:
                    state.setdefault("l2q", []).append(do_l2)

            def flush_l2():
                q = state.get("l2q", [])
                if q:
                    q.pop(0)()

            def finish_l2(ps2a, ps2b, ps2w, g, mq):
                if probe == "pe_only":
                    return
                if mq == 31:
                    if wide:
                        oa = outp.tile([T, 1024], f32, tag="oa")
                        nc.scalar.activation(
                            oa[:], ps2w[:, 0:1024], Act.Identity,
                            bias=b2t[:, g : g + 1],
                        )
                        nc.sync.dma_start(
                            out_d[128 * g : 128 * (g + 1), 0:1024], oa[:]
                        )
                    else:
                        oa = outp.tile([T, 512], f32, tag="oa")
                        nc.scalar.activation(
                            oa[:], ps2a, Act.Identity, bias=b2t[:, g : g + 1]
                        )
                        nc.sync.dma_start(out_d[128 * g : 128 * (g + 1), 0:512], oa[:])
                        ob = outp.tile([T, 512], f32, tag="ob")
                        nc.scalar.activation(
                            ob[:], ps2b, Act.Identity, bias=b2t[:, g : g + 1]
                        )
                        nc.sync.dma_start(
                            out_d[128 * g : 128 * (g + 1), 512:1024], ob[:]
                        )

            def body(_iv=None):
                S = int(skew)
                D = int(l2_delay)
                for m in range(M_TILES + S):
                    if flush_first and D and m >= S + D:
                        flush_l2()
                    if m < M_TILES:
                        stage_l0(m)
                    if m >= S:
                        stage_l12(m - S)
                    if not flush_first and D and m >= S + D:
                        flush_l2()
                while state.get("l2q"):
                    flush_l2()

            if probe == "pe_only":
                zpad = cp.tile([T, 16], f32, tag="zpad")
                nc.gpsimd.memset(zpad[:], 0.0)
                nc.sync.dma_start(out_d[0:T, 0:16], zpad[:])

            if loop_R is None:
                for _ in range(repeat):
                    body()
            else:
                with tc.For_i(0, loop_R, 1) as iv:
                    body(iv)

    _split_sync_waits(nc, cap=wait_cap)
    return nc


def prep_inputs(x, w0, b0, w1, b1, w2, b2, h_dtype="bf16", pe_bias=False):
    """Host-side reshuffle of the full inputs into the per-core tensors."""
    import ml_dtypes

    mixed = h_dtype == "mixed"
    bf = ml_dtypes.bfloat16 if h_dtype == "bf16" else np.float32
    x = np.ascontiguousarray(np.asarray(x, dtype=np.float32))
    w0 = np.asarray(w0, dtype=np.float32)
    b0 = np.asarray(b0, dtype=np.float32)
    w1 = np.asarray(w1, dtype=np.float32)
    b1 = np.asarray(b1, dtype=np.float32)
    w2 = np.asarray(w2, dtype=np.float32)
    b2 = np.asarray(b2, dtype=np.float32)

    # L0 stationaries: mask self-loop; [j, (m p)] with column 128m+p -> ti
    w0m = w0.copy()
    w0m[np.arange(T), :, np.arange(T)] = 0.0
    w0w = np.ascontiguousarray(w0m.transpose(2, 0, 1).reshape(T, T * H))

    # L1 stationaries: block-diag of the pair's transposed weights (bf16)
    w1T = w1.transpose(0, 2, 1)  # [t, i_in, i_out]
    w1s = np.zeros((M_TILES, T, T), np.float32)
    w1s[:, :H, :H] = w1T[0::2]
    w1s[:, H:, H:] = w1T[1::2]
    w1w = np.ascontiguousarray(
        w1s.transpose(1, 0, 2).reshape(T, M_TILES * T).astype(bf)
    )

    # L2 stationaries: pair m owns columns 4*(m%32) .. +4 (bf16)
    w2T = w2.transpose(0, 2, 1)  # [t, i, c]
    w2s = np.zeros((M_TILES, T, T), np.float32)
    for m in range(M_TILES):
        col = 4 * (m % 32)
        w2s[m, :H, col : col + C] = w2T[2 * m]
        w2s[m, H:, col + C : col + 2 * C] = w2T[2 * m + 1]
    w2w = np.ascontiguousarray(
        w2s.transpose(1, 0, 2).reshape(T, M_TILES * T).astype(bf)
    )

    b0s = np.ascontiguousarray(b0.reshape(-1).reshape(M_TILES, T).T)
    b1s = np.ascontiguousarray(b1.reshape(-1).reshape(M_TILES, T).T)
    b2s = np.ascontiguousarray(b2.reshape(-1).reshape(2, T).T)

    shared = {
        "w0w": w0w, "w1w": w1w, "w2w": w2w,
        "b0s": b0s, "b1s": b1s, "b2s": b2s,
    }
    if mixed:
        shared["w1wb"] = np.ascontiguousarray(w1w.astype(ml_dtypes.bfloat16))
        shared["w2wb"] = np.ascontiguousarray(w2w.astype(ml_dtypes.bfloat16))
    if pe_bias:
        # bias rows for the PE-bias injection path
        shared["b0r"] = np.ascontiguousarray(b0.reshape(1, -1))
        shared["b1r"] = np.ascontiguousarray(b1.reshape(1, -1))
    in_maps = []
    for c in range(N_CORES):
        xt_c = np.ascontiguousarray(x[c * BC : (c + 1) * BC].T)  # [128, BC]
        in_maps.append({"xt": xt_c, **shared})
    return in_maps


def gather_output(results):
    """results: list of per-core {"out": [256, BC]} -> full [B, T, C]."""
    parts = []
    for c in range(N_CORES):
        o = np.asarray(results[c]["out"])  # [2T, BC], row r = t*2+c
        parts.append(o.reshape(T, C, BC).transpose(2, 0, 1))
    return np.ascontiguousarray(np.concatenate(parts, axis=0))


_NC_CACHE = {}

BEST_CONFIG = dict(l2_delay=1, h_bufs=4, flush_first=True)


def kernel(x, w0, b0, w1, b1, w2, b2):
    from concourse.bass_utils import run_bass_kernel_spmd

    if "nc" not in _NC_CACHE:
        _NC_CACHE["nc"] = build_program(**BEST_CONFIG)
    nc = _NC_CACHE["nc"]
    in_maps = prep_inputs(
        x, w0, b0, w1, b1, w2, b2, h_dtype=BEST_CONFIG.get("h_dtype", "bf16")
    )
    res = run_bass_kernel_spmd(nc, in_maps, core_ids=list(range(N_CORES)))
    return gather_output(res.results)

